# revision 1
# baseline (speedup 1.0000x reference)
"""Trainium2 Bass kernel for CommunityPreservationLoss (triplet margin loss
over pairwise distances with hardest-negative mining).

Strategy (8 NeuronCores, SPMD):
  - Shard anchor rows: 1024 rows/core = 8 blocks of 128 anchors.
  - Per block, the PE assembles d2 = sq_i + sq_j - 2*x_i.x_j directly in PSUM
    with bf16 matmuls: the fp32 operand -2*X is split into bf16 hi+lo, and
    x.x' is computed as hi@hi + hi@lo + lo@hi (the lo@lo term is ~2^-18 and
    dropped); sq_j is added via a K=3 ones @ [sq_hi;sq_mid;sq_lo] matmul and
    sq_i rides the per-partition bias of the sqrt evacuation.
  - ACT evacuates PSUM with dist = Sqrt(psum + sq_i).
  - DVE tensor_scalar builds maskshift = -64 * (comm_j != comm_i)  (bf16).
  - DVE tensor_tensor adds it in place: md = dist + maskshift. Different-
    community pairs sit at dist-64 (all negative), same-community pairs at
    dist (>0), so row-min(md) + 64 = hardest-negative distance.
  - tensor_reduce(min) extracts the row min (VectorE or GpSimd, see
    REDUCE_ENGINE).
  - ACT computes Relu(md + (margin - minneg)) with a fused row-sum
    (accum_out): same-community pairs contribute the triplet margin, the
    diagonal and different-community pairs relu to zero.
  - Host sums the 8x[128,8] partials (f64) and divides by the exact triplet
    count from a bincount of communities.

The diagonal is handled by inflating the anchor-side sq by 2e-3: d2_ii stays
positive (no sqrt NaN), dist_ii ~ 0.05 which is excluded from the min (diff
pairs are < 0) and from the pos-sum (0.05 - c < 0 for this data regime).
"""

import numpy as np
import ml_dtypes

BF16 = ml_dtypes.bfloat16

N = 8192          # nodes
D = 128           # embedding dim
NCORES = 8
RPC = N // NCORES  # rows per core = 1024
NBLK = RPC // 128  # anchor blocks per core = 8
GRP = 2048         # psum tile width (4 banks)
SUB = 512          # matmul moving width
MARGIN = 1.0
SHIFT = 64.0       # additive mask shift for different-community pairs
DIAG_EPS = 2e-3    # anchor-side sq inflation (keeps diagonal d2 > 0)

REDUCE_ENGINE = "vector"   # free-dim reduce is VectorE-only

_cache = {}


def _build_nc():
    import concourse.tile as tile
    from concourse import bacc, mybir

    f32 = mybir.dt.float32
    bf16 = mybir.dt.bfloat16
    AF = mybir.ActivationFunctionType
    OP = mybir.AluOpType

    nc = bacc.Bacc("TRN2", target_bir_lowering=False, debug=False)

    xth_d = nc.dram_tensor("xth", [D, N], bf16, kind="ExternalInput").ap()
    xtl_d = nc.dram_tensor("xtl", [D, N], bf16, kind="ExternalInput").ap()
    m2h_d = nc.dram_tensor("m2h", [D, RPC], bf16, kind="ExternalInput").ap()
    m2l_d = nc.dram_tensor("m2l", [D, RPC], bf16, kind="ExternalInput").ap()
    sqr_d = nc.dram_tensor("sqr", [3, N], bf16, kind="ExternalInput").ap()
    one_d = nc.dram_tensor("one", [3, D], bf16, kind="ExternalInput").ap()
    sqa_d = nc.dram_tensor("sqa", [128, NBLK], f32, kind="ExternalInput").ap()
    cmb_d = nc.dram_tensor("cmb", [128, N], bf16, kind="ExternalInput").ap()
    cma_d = nc.dram_tensor("cma", [128, NBLK], f32, kind="ExternalInput").ap()
    out_d = nc.dram_tensor("possum", [128, NBLK], f32, kind="ExternalOutput").ap()

    with tile.TileContext(nc) as tc:
        with (
            tc.tile_pool(name="const", bufs=1) as constp,
            tc.tile_pool(name="dist", bufs=2) as distp,
            tc.tile_pool(name="msk", bufs=2) as mskp,
            tc.tile_pool(name="small", bufs=4) as smallp,
            tc.tile_pool(name="ps", bufs=2, space="PSUM") as psp,
        ):
            xth_s = constp.tile([D, N], bf16, tag="xth")
            xtl_s = constp.tile([D, N], bf16, tag="xtl")
            m2h_s = constp.tile([D, RPC], bf16, tag="m2h")
            m2l_s = constp.tile([D, RPC], bf16, tag="m2l")
            sqr_s = constp.tile([3, N], bf16, tag="sqr")
            one_s = constp.tile([3, D], bf16, tag="one")
            sqa_s = constp.tile([128, NBLK], f32, tag="sqa")
            cmb_s = constp.tile([128, N], bf16, tag="cmb")
            cma_s = constp.tile([128, NBLK], f32, tag="cma")
            possum_s = constp.tile([128, NBLK], f32, tag="possum")

            # small operands first so block-0 matmuls can start ASAP,
            # then xt chunks in compute order, then the mask operands
            nc.sync.dma_start(out=m2h_s[:], in_=m2h_d[:])
            nc.sync.dma_start(out=m2l_s[:], in_=m2l_d[:])
            nc.sync.dma_start(out=sqr_s[:], in_=sqr_d[:])
            nc.sync.dma_start(out=one_s[:], in_=one_d[:])
            nc.sync.dma_start(out=sqa_s[:], in_=sqa_d[:])
            nc.sync.dma_start(out=cma_s[:], in_=cma_d[:])
            for g in range(N // GRP):
                gs = slice(g * GRP, (g + 1) * GRP)
                nc.sync.dma_start(out=xth_s[:, gs], in_=xth_d[:, gs])
                nc.sync.dma_start(out=xtl_s[:, gs], in_=xtl_d[:, gs])
            nc.sync.dma_start(out=cmb_s[:], in_=cmb_d[:])

            def emit_front(b):
                """mask + matmuls + sqrt-evacuation for block b."""
                bsl = slice(b, b + 1)
                blk = slice(b * 128, (b + 1) * 128)
                # maskshift = (comm_j != comm_i) * -SHIFT   (0 / -64, exact bf16)
                # (walrus rejects TensorScalarPtr on Pool, so this stays on
                # VectorE; bf16 runs at 4x there.)
                msk = mskp.tile([128, N], bf16, tag="msk")
                nc.vector.tensor_scalar(
                    out=msk[:],
                    in0=cmb_s[:],
                    scalar1=cma_s[:, bsl],
                    scalar2=-SHIFT,
                    op0=OP.not_equal,
                    op1=OP.mult,
                )

                dist = distp.tile([128, N], f32, tag="dist")
                for g in range(N // GRP):
                    ps = psp.tile([128, GRP], f32, tag="ps")
                    nsub = GRP // SUB
                    # weight-reuse order: (m2h x2 passes), m2l, ones
                    for s in range(nsub):
                        c0 = g * GRP + s * SUB
                        nc.tensor.matmul(
                            ps[:, s * SUB:(s + 1) * SUB],
                            m2h_s[:, blk], xth_s[:, c0:c0 + SUB],
                            start=True, stop=False,
                        )
                    for s in range(nsub):
                        c0 = g * GRP + s * SUB
                        nc.tensor.matmul(
                            ps[:, s * SUB:(s + 1) * SUB],
                            m2h_s[:, blk], xtl_s[:, c0:c0 + SUB],
                            start=False, stop=False,
                        )
                    for s in range(nsub):
                        c0 = g * GRP + s * SUB
                        nc.tensor.matmul(
                            ps[:, s * SUB:(s + 1) * SUB],
                            m2l_s[:, blk], xth_s[:, c0:c0 + SUB],
                            start=False, stop=False,
                        )
                    for s in range(nsub):
                        c0 = g * GRP + s * SUB
                        nc.tensor.matmul(
                            ps[:, s * SUB:(s + 1) * SUB],
                            one_s[:], sqr_s[:, c0:c0 + SUB],
                            start=False, stop=True,
                        )
                    # dist = sqrt(psum + sq_i)
                    nc.scalar.activation(
                        dist[:, g * GRP:(g + 1) * GRP],
                        ps[:],
                        AF.Sqrt,
                        bias=sqa_s[:, bsl],
                        scale=1.0,
                    )
                return dist, msk

            def emit_back(b, dist, msk):
                """md = dist + maskshift (in place), row-min, margin+sum."""
                bsl = slice(b, b + 1)
                mn = smallp.tile([128, 1], f32, tag="mn")
                nc.vector.tensor_tensor(
                    out=dist[:], in0=dist[:], in1=msk[:], op=OP.add
                )
                nc.vector.tensor_reduce(
                    out=mn[:], in_=dist[:], op=OP.min, axis=mybir.AxisListType.X
                )
                # bias = margin - minneg = MARGIN - (mn + SHIFT)
                cbias = smallp.tile([128, 1], f32, tag="cb")
                nc.vector.tensor_scalar(
                    out=cbias[:],
                    in0=mn[:],
                    scalar1=-1.0,
                    scalar2=MARGIN - SHIFT,
                    op0=OP.mult,
                    op1=OP.add,
                )
                # possum[:, b] = sum_j relu(md + bias)
                nc.scalar.activation(
                    msk[:],
                    dist[:],
                    AF.Relu,
                    bias=cbias[:],
                    scale=1.0,
                    accum_out=possum_s[:, bsl],
                )

            # software pipeline: block b's post-processing is emitted after
            # block b+1's matmuls/evacs, so PSUM evacuations never queue
            # behind the big ACT2 on the Scalar engine and the PE stays hot.
            pend = None
            for b in range(NBLK):
                front = emit_front(b)
                if pend is not None:
                    emit_back(b - 1, *pend)
                pend = front
            emit_back(NBLK - 1, *pend)

            nc.sync.dma_start(out=out_d[:], in_=possum_s[:])

    nc.compile()
    return nc


def get_nc():
    if "nc" not in _cache:
        _cache["nc"] = _build_nc()
    return _cache["nc"]


def _split_lo(v32):
    """v32 (f32) -> (hi, lo) bf16 arrays with hi + lo ~ v32 (2^-16 rel)."""
    h = v32.astype(BF16)
    lo = (v32 - h.astype(np.float32)).astype(BF16)
    return h, lo


def make_in_maps(embeddings, communities):
    X = np.ascontiguousarray(np.asarray(embeddings, dtype=np.float32))
    comm = np.asarray(communities).astype(np.int64)
    assert X.shape == (N, D) and comm.shape == (N,)

    sq64 = (X.astype(np.float64) ** 2).sum(axis=1)
    sq = sq64.astype(np.float32)
    sqa_full = (sq64 + DIAG_EPS).astype(np.float32)
    commf = comm.astype(np.float32)

    xt = np.ascontiguousarray(X.T)                       # [128, 8192] f32
    xth, xtl = _split_lo(xt)
    # sq -> 3-way bf16 split (exact to ~2^-24 rel)
    sqh = sq.astype(BF16)
    r = sq - sqh.astype(np.float32)
    sqm = r.astype(BF16)
    sql = (r - sqm.astype(np.float32)).astype(BF16)
    sqr = np.ascontiguousarray(np.stack([sqh, sqm, sql], axis=0))  # [3, N] bf16
    one = np.ones((3, D), dtype=BF16)
    cmb = np.ascontiguousarray(
        np.broadcast_to(commf[None, :], (128, N))
    ).astype(BF16)

    in_maps = []
    for c in range(NCORES):
        rows = slice(c * RPC, (c + 1) * RPC)
        m2 = np.ascontiguousarray((-2.0 * X[rows]).T)    # [128, 1024] f32
        m2h, m2l = _split_lo(m2)
        sqa = np.ascontiguousarray(sqa_full[rows].reshape(NBLK, 128).T)
        cma = np.ascontiguousarray(commf[rows].reshape(NBLK, 128).T)
        in_maps.append(
            dict(xth=np.ascontiguousarray(xth), xtl=np.ascontiguousarray(xtl),
                 m2h=np.ascontiguousarray(m2h), m2l=np.ascontiguousarray(m2l),
                 sqr=sqr, one=one, sqa=sqa, cmb=cmb, cma=cma)
        )
    return in_maps, comm


def finalize(results, comm):
    """results: list (per core) of dicts with 'possum' [128, NBLK] f32."""
    total = 0.0
    for r in results:
        total += float(r["possum"].astype(np.float64).sum())
    counts = np.bincount(comm)
    counts = counts[counts < N]  # rows with no negative are invalid
    cnt = int((counts * (counts - 1)).sum())
    if cnt == 0:
        return np.array(0.0, dtype=np.float32)
    loss = np.float32(total) / np.float32(cnt)
    return np.array(loss, dtype=np.float32)


def kernel(embeddings, communities):
    from concourse.bass_utils import run_bass_kernel_spmd

    nc = get_nc()
    in_maps, comm = make_in_maps(embeddings, communities)
    res = run_bass_kernel_spmd(nc, in_maps, core_ids=list(range(NCORES)))
    return finalize(res.results, comm)

